# revision 12
# baseline (speedup 1.0000x reference)
"""MoE (5 experts, 3->20->3, softmax gate) over B=1M tokens on 8 trn2 cores.

Math: per token x (R^3):
  s[e,j]   = W1[e,:,j].x + b1[e,j]
  h[e,j]   = relu(s[e,j])
  eo[e,o]  = W2[e,:,o].h[e] + b2[e,o]
  g[e]     = softmax(x.Wg + bg)[e]
  mixed[o] = sum_e g[e] * eo[e,o]
Returns (mixed [B,3], gate [B,5]).

Device dataflow (per core, 131072 tokens, data-parallel over 8 cores):
  token-major gate path (tokens on partitions, 16K-token chunks):
    z = x.Wg + bg (DVE/GPSIMD), p = exp(z) (ACT), S = sum_e p (DVE),
    g = p/S (DVE) -> gate output (contiguous store)
    xgg[r] = {x_i * g_e (r=3e+i), g_e (r=15+e)}  (DVE)
  transpose xgg to feature-major [20, tokens] via a DRAM round-trip (SBUF
  DMA access patterns cannot cross partitions, so stage out token-major and
  load back with a transposed DRAM access pattern; inner runs stay 512B)
  feature-major expert path (tokens on free dim, 512-token tiles):
    mm1: psum[105,512] = L1^T @ xgg^T   (L1 folds W1, b1, g-passthrough;
         rows 20e+j hold g_e*s[e,j], rows 100+e hold g_e)
    transit: hg = relu(psum)  (ACT/DVE alternating; relu(g*s)=g*relu(s), g>0)
    mm2: mixed[3,512] = L2^T @ hg       (L2 folds W2 and b2*g)
  mixed tiles stack 3-deep per PSUM bank at bases 0/32/64 (matmul output
  base partition must be one of those) -> one [67,512] copy per 3 tiles;
  host de-interleaves the staged stacks.
"""

import sys

sys.path.insert(0, "/opt/trn_rl_repo")

import numpy as np

import concourse.bacc as bacc
import concourse.mybir as mybir
from concourse.tile import TileContext
from concourse.bass_utils import run_bass_kernel_spmd

F32 = mybir.dt.float32
AF = mybir.ActivationFunctionType
ALU = mybir.AluOpType

B = 1 << 20
NCORES = 8
E, IN, HID, OUT = 5, 3, 20, 3
TILE = 512
CHUNK = 16384
TPP = CHUNK // 128  # tokens per partition per chunk = 128
TPC = CHUNK // TILE  # tiles per chunk = 32

_NC_CACHE = {}


def build_nc(bc):
    """Build + compile the per-core Bass program for bc tokens."""
    nchunk = bc // CHUNK
    assert nchunk * CHUNK == bc

    nc = bacc.Bacc(None)
    x = nc.dram_tensor("x", [bc, IN], F32, kind="ExternalInput")
    cbd = nc.dram_tensor("cb", [128, 256], F32, kind="ExternalInput")
    ngrp = (TPC + 2) // 3  # mm2 groups of 3 tiles per PSUM bank
    gate = nc.dram_tensor("gate", [bc, E], F32, kind="ExternalOutput")
    mx = nc.dram_tensor("mx", [nchunk, ngrp, 3, 3, TILE], F32, kind="ExternalOutput")
    xgd = nc.dram_tensor("xgd", [nchunk, 128, 15 + E, TPP], F32)  # staging

    with TileContext(nc) as tc:
        with (
            tc.tile_pool(name="consts", bufs=1) as cpool,
            tc.tile_pool(name="xin", bufs=2) as xpool,
            tc.tile_pool(name="gwork", bufs=2) as gpool,
            tc.tile_pool(name="xgg", bufs=2) as xggpool,
            tc.tile_pool(name="xt", bufs=3) as tpool,
            tc.tile_pool(name="hg", bufs=4) as hpool,
            tc.tile_pool(name="mst", bufs=2) as mpool,
            tc.tile_pool(name="psh", bufs=3, space="PSUM") as php,
            tc.tile_pool(name="psm", bufs=2, space="PSUM") as pmp,
        ):
            cb = cpool.tile([128, 256], F32)
            nc.sync.dma_start(out=cb, in_=cbd[:, :])
            L1 = cb[0:20, 0:105]
            L2w = cb[0:105, 105:172]  # [105,67]: col 0..2 real, rest zero
            L2 = cb[0:105, 105:108]
            WgR = cb[:, 172:187].rearrange("p (i e) -> p i e", i=IN)  # [128,3,5]
            bgR = cb[:, 187:192]  # [128,5]

            for c in range(nchunk):
                t0 = c * CHUNK
                # ---- token-major gate path ----
                xc = xpool.tile([128, TPP * IN], F32)
                nc.sync.dma_start(
                    out=xc,
                    in_=x[t0 : t0 + CHUNK, :].rearrange("(p t) i -> p (t i)", p=128),
                )
                xcv = xc.rearrange("p (t i) -> p t i", i=IN)  # [128,TPP,3]

                def xib(i):  # x_i broadcast over e: [128,TPP,5]
                    return xcv[:, :, i : i + 1].broadcast_to([128, TPP, E])

                def wgb(i):  # Wg[i,:] broadcast over t: [128,TPP,5]
                    return WgR[:, i, :][:, None, :].broadcast_to([128, TPP, E])

                z = gpool.tile([128, TPP, E], F32, tag="z")
                tmp = gpool.tile([128, TPP, E], F32, tag="tmp")
                nc.gpsimd.tensor_mul(z, xib(0), wgb(0))
                nc.gpsimd.tensor_mul(tmp, xib(1), wgb(1))
                nc.gpsimd.tensor_add(z, z, tmp)
                nc.gpsimd.tensor_mul(tmp, xib(2), wgb(2))
                nc.gpsimd.tensor_add(z, z, tmp)
                nc.gpsimd.tensor_add(
                    z, z, bgR[:, None, :].broadcast_to([128, TPP, E])
                )
                pexp = gpool.tile([128, TPP, E], F32, tag="pexp")
                nc.scalar.activation(pexp, z, AF.Exp)
                S = gpool.tile([128, TPP], F32, tag="S")
                nc.vector.tensor_reduce(S, pexp, axis=mybir.AxisListType.X, op=ALU.add)
                Sr = gpool.tile([128, TPP], F32, tag="Sr")
                nc.vector.reciprocal(Sr, S)
                g = gpool.tile([128, TPP, E], F32, tag="g")
                nc.vector.tensor_mul(
                    g, pexp, Sr[:, :, None].broadcast_to([128, TPP, E])
                )
                nc.sync.dma_start(
                    out=gate[t0 : t0 + CHUNK, :].rearrange("(p t) e -> p t e", p=128),
                    in_=g,
                )
                # xgg rows: r=3e+i -> x_i*g_e ; r=15+e -> g_e   [128, 20, TPP]
                xgg = xggpool.tile([128, 15 + E, TPP], F32)
                nc.vector.tensor_mul(
                    xgg[:, 0:15, :].rearrange("p (e i) t -> p e i t", e=E),
                    xcv.rearrange("p t i -> p i t")[:, None, :, :].broadcast_to(
                        [128, E, IN, TPP]
                    ),
                    g.rearrange("p t e -> p e t")[:, :, None, :].broadcast_to(
                        [128, E, IN, TPP]
                    ),
                )
                nc.vector.tensor_copy(
                    xgg[:, 15 : 15 + E, :], g.rearrange("p t e -> p e t")
                )
                # ---- transpose to feature-major via DRAM round-trip ----
                nc.sync.dma_start(out=xgd[c], in_=xgg)
                xth = []
                for q in range(4):  # quarter-chunk transposed loads [20, 4096]
                    xt = tpool.tile([20, CHUNK // 4], F32)
                    nc.sync.dma_start(
                        out=xt.rearrange("r (p t) -> r p t", p=32),
                        in_=xgd[c, 32 * q : 32 * q + 32, :, :].rearrange(
                            "p r t -> r p t"
                        ),
                    )
                    xth.append(xt)
                # ---- feature-major expert path ----
                for grp in range(ngrp):
                    u_lo = 3 * grp
                    u_hi = min(u_lo + 3, TPC)
                    psumM = pmp.tile([67, TILE], F32)
                    hgs = []
                    # group matmuls: all mm1s first, then all mm2s, so the
                    # L1/L2 stationary reloads amortize over the group
                    for u in range(u_lo, u_hi):
                        rhs = xth[u // 8][:, (u % 8) * TILE : (u % 8 + 1) * TILE]
                        psumH = php.tile([105, TILE], F32)
                        nc.tensor.matmul(
                            psumH, lhsT=L1, rhs=rhs, start=True, stop=True
                        )
                        hg = hpool.tile([105, TILE], F32)
                        if u % 3 < 2:
                            nc.scalar.activation(hg, psumH, AF.Relu)
                        else:
                            nc.vector.tensor_scalar_max(hg, psumH, 0.0)
                        hgs.append(hg)
                    for slot, hg in enumerate(hgs):
                        # slot 0 writes all 67 rows (zero-padded L2 columns)
                        # so the bank is fully initialized for the copy below
                        if slot == 0:
                            nc.tensor.matmul(
                                psumM, lhsT=L2w, rhs=hg, start=True, stop=True
                            )
                        else:
                            nc.tensor.matmul(
                                psumM[32 * slot : 32 * slot + 3, :],
                                lhsT=L2,
                                rhs=hg,
                                start=True,
                                stop=True,
                            )
                    mst = mpool.tile([67, TILE], F32)
                    if grp % 2 == 0:
                        nc.scalar.activation(mst, psumM, AF.Copy)
                    else:
                        nc.vector.tensor_copy(mst, psumM)
                    for slot in range(len(hgs)):
                        nc.sync.dma_start(
                            out=mx[c, grp, slot, :, :],
                            in_=mst[32 * slot : 32 * slot + 3, :],
                        )
    nc.compile()
    return nc


def make_const_blob(W1, b1, W2, b2, Wg, bg):
    cb = np.zeros((128, 256), np.float32)
    for e in range(E):
        for i in range(IN):
            cb[3 * e + i, 20 * e : 20 * e + 20] = W1[e, i, :]
        cb[15 + e, 20 * e : 20 * e + 20] = b1[e, :]
        cb[15 + e, 100 + e] = 1.0
        cb[20 * e : 20 * e + 20, 105:108] = W2[e, :, :]
        cb[100 + e, 105:108] = b2[e, :]
    for i in range(IN):
        for e in range(E):
            cb[:, 172 + 5 * i + e] = Wg[i, e]
    cb[:, 187:192] = bg
    return cb


def assemble_mixed(mx):
    """mx [nchunk, ngrp, 3, 3, TILE] -> mixed rows [nchunk*CHUNK, 3].

    mx[c, grp, slot, o, n] is mixed component o of token c*CHUNK +
    (3*grp+slot)*TILE + n; slots past TPC are padding."""
    nchunk, ngrp = mx.shape[0], mx.shape[1]
    m = mx.reshape(nchunk, ngrp * 3, 3, TILE)[:, :TPC]  # [c, u, o, n]
    return m.transpose(0, 1, 3, 2).reshape(nchunk * CHUNK, 3)


def kernel(x, W1, b1, W2, b2, Wg, bg):
    x = np.ascontiguousarray(np.asarray(x, np.float32))
    bc = x.shape[0] // NCORES
    if bc not in _NC_CACHE:
        _NC_CACHE[bc] = build_nc(bc)
    nc = _NC_CACHE[bc]
    cb = make_const_blob(
        np.asarray(W1), np.asarray(b1), np.asarray(W2), np.asarray(b2),
        np.asarray(Wg), np.asarray(bg),
    )
    in_maps = [
        {"x": x[c * bc : (c + 1) * bc], "cb": cb} for c in range(NCORES)
    ]
    res = run_bass_kernel_spmd(nc, in_maps, list(range(NCORES)))
    gate = np.concatenate([res.results[c]["gate"] for c in range(NCORES)], axis=0)
    mixed = np.concatenate(
        [assemble_mixed(res.results[c]["mx"]) for c in range(NCORES)], axis=0
    )
    return mixed, gate


# revision 19
# speedup vs baseline: 6264.3822x; 6264.3822x over previous
"""MoE (5 experts, 3->20->3, softmax gate) over B=1M tokens on 8 trn2 cores.

Math: per token x (R^3):
  s[e,j]   = W1[e,:,j].x + b1[e,j]
  h[e,j]   = relu(s[e,j])
  eo[e,o]  = W2[e,:,o].h[e] + b2[e,o]
  g[e]     = softmax(x.Wg + bg)[e]
  mixed[o] = sum_e g[e] * eo[e,o]
Returns (mixed [B,3], gate [B,5]).

Device dataflow (per core, 131072 tokens, data-parallel over 8 cores):
  token-major gate path (tokens on partitions, 16K-token chunks):
    z = x.Wg + bg (DVE/GPSIMD), p = exp(z) (ACT), S = sum_e p (DVE),
    g = p/S (DVE) -> gate output (contiguous store)
    xgg[r] = {x_i * g_e (r=3e+i), g_e (r=15+e)}  (DVE)
  transpose xgg to feature-major [20, tokens] via a DRAM round-trip (SBUF
  DMA access patterns cannot cross partitions, so stage out token-major and
  load back with a transposed DRAM access pattern; inner runs stay 512B)
  feature-major expert path (tokens on free dim, 512-token tiles):
    mm1: psum[105,512] = L1^T @ xgg^T   (L1 folds W1, b1, g-passthrough;
         rows 20e+j hold g_e*s[e,j], rows 100+e hold g_e)
    transit: hg = relu(psum)  (ACT/DVE alternating; relu(g*s)=g*relu(s), g>0)
    mm2: mixed[3,512] = L2^T @ hg       (L2 folds W2 and b2*g)
    mm2: four sub-matmuls per tile with the hg 128-token slice as the
         STATIONARY operand (lhsT=hg[:,128q:], rhs=L2 [105,3]) so mixed
         lands token-major [128,3] at PSUM base 0 (f32r matmuls may only
         write base 0); a whole chunk's mixed packs into one PSUM bank
         [128, 384] -> one copy + one contiguous store per 32 tiles.
"""

import sys

sys.path.insert(0, "/opt/trn_rl_repo")

import numpy as np

import concourse.bacc as bacc
import concourse.mybir as mybir
from concourse.tile import TileContext
from concourse.bass_utils import run_bass_kernel_spmd

F32 = mybir.dt.float32
F32R = mybir.dt.float32r
AF = mybir.ActivationFunctionType
ALU = mybir.AluOpType

B = 1 << 20
NCORES = 8
E, IN, HID, OUT = 5, 3, 20, 3
TILE = 512
CHUNK = 16384
TPP = CHUNK // 128  # tokens per partition per chunk = 128
TPC = CHUNK // TILE  # tiles per chunk = 32

_NC_CACHE = {}


def build_nc(bc):
    """Build + compile the per-core Bass program for bc tokens."""
    nchunk = bc // CHUNK
    assert nchunk * CHUNK == bc

    nc = bacc.Bacc(None)
    x = nc.dram_tensor("x", [bc, IN], F32, kind="ExternalInput")
    cbd = nc.dram_tensor("cb", [128, 256], F32, kind="ExternalInput")
    cbrd = nc.dram_tensor("cbr", [105, 172], F32R, kind="ExternalInput")
    gate = nc.dram_tensor("gate", [bc, E], F32, kind="ExternalOutput")
    mx = nc.dram_tensor("mx", [nchunk, 128, 16 * TPC], F32, kind="ExternalOutput")
    xgd = nc.dram_tensor("xgd", [nchunk, 128, 15 + E, TPP], F32R)  # staging

    with TileContext(nc) as tc:
        with (
            tc.tile_pool(name="consts", bufs=1) as cpool,
            tc.tile_pool(name="xin", bufs=3) as xpool,
            tc.tile_pool(name="gwork", bufs=3) as gpool,
            tc.tile_pool(name="xgg", bufs=3) as xggpool,
            tc.tile_pool(name="xt", bufs=4) as tpool,
            tc.tile_pool(name="hg", bufs=6) as hpool,
            tc.tile_pool(name="mst", bufs=2) as mpool,
            tc.tile_pool(name="psh", bufs=4, space="PSUM") as php,
            tc.tile_pool(name="psm", bufs=3, space="PSUM") as pmp,
        ):
            cb = cpool.tile([128, 256], F32)
            nc.sync.dma_start(out=cb, in_=cbd[:, :])
            cbr = cpool.tile([105, 172], F32R)
            nc.sync.dma_start(out=cbr, in_=cbrd[:, :])
            L1 = cbr[0:20, 0:105]
            L2p = cbr[0:105, 105:109]  # [105,4]: col 3 zero (f32r needs even n)
            WgR = cb[:, 172:187].rearrange("p (i e) -> p i e", i=IN)  # [128,3,5]
            bgR = cb[:, 187:192]  # [128,5]

            for c in range(nchunk):
                t0 = c * CHUNK
                # ---- token-major gate path ----
                xc = xpool.tile([128, TPP * IN], F32)
                nc.sync.dma_start(
                    out=xc,
                    in_=x[t0 : t0 + CHUNK, :].rearrange("(p t) i -> p (t i)", p=128),
                )
                xcv = xc.rearrange("p (t i) -> p t i", i=IN)  # [128,TPP,3]

                def xib(i):  # x_i broadcast over e: [128,TPP,5]
                    return xcv[:, :, i : i + 1].broadcast_to([128, TPP, E])

                def wgb(i):  # Wg[i,:] broadcast over t: [128,TPP,5]
                    return WgR[:, i, :][:, None, :].broadcast_to([128, TPP, E])

                z = gpool.tile([128, TPP, E], F32, tag="z")
                tmp = gpool.tile([128, TPP, E], F32, tag="tmp")
                nc.vector.tensor_mul(z, xib(0), wgb(0))
                nc.vector.tensor_mul(tmp, xib(1), wgb(1))
                nc.vector.tensor_add(z, z, tmp)
                nc.vector.tensor_mul(tmp, xib(2), wgb(2))
                nc.vector.tensor_add(z, z, tmp)
                nc.vector.tensor_add(
                    z, z, bgR[:, None, :].broadcast_to([128, TPP, E])
                )
                pexp = gpool.tile([128, TPP, E], F32, tag="pexp")
                nc.scalar.activation(pexp, z, AF.Exp)
                S = gpool.tile([128, TPP], F32, tag="S")
                nc.vector.tensor_reduce(S, pexp, axis=mybir.AxisListType.X, op=ALU.add)
                Sr = gpool.tile([128, TPP], F32, tag="Sr")
                nc.vector.reciprocal(Sr, S)
                g = gpool.tile([128, TPP, E], F32, tag="g")
                nc.vector.tensor_mul(
                    g, pexp, Sr[:, :, None].broadcast_to([128, TPP, E])
                )
                nc.scalar.dma_start(
                    out=gate[t0 : t0 + CHUNK, :].rearrange("(p t) e -> p t e", p=128),
                    in_=g,
                )
                # xgg rows: r=3e+i -> x_i*g_e ; r=15+e -> g_e   [128, 20, TPP]
                xgg = xggpool.tile([128, 15 + E, TPP], F32R)
                nc.vector.tensor_mul(
                    xgg[:, 0:15, :].rearrange("p (e i) t -> p e i t", e=E),
                    xcv.rearrange("p t i -> p i t")[:, None, :, :].broadcast_to(
                        [128, E, IN, TPP]
                    ),
                    g.rearrange("p t e -> p e t")[:, :, None, :].broadcast_to(
                        [128, E, IN, TPP]
                    ),
                )
                nc.vector.tensor_copy(
                    xgg[:, 15 : 15 + E, :], g.rearrange("p t e -> p e t")
                )
                # ---- transpose to feature-major via DRAM round-trip ----
                nc.scalar.dma_start(out=xgd[c], in_=xgg)
                xth = []
                for q in range(4):  # quarter-chunk transposed loads [20, 4096]
                    xt = tpool.tile([20, CHUNK // 4], F32R)
                    nc.gpsimd.dma_start(
                        out=xt.rearrange("r (p t) -> r p t", p=32),
                        in_=xgd[c, 32 * q : 32 * q + 32, :, :].rearrange(
                            "p r t -> r p t"
                        ),
                    )
                    xth.append(xt)
                # ---- feature-major expert path ----
                psumM = pmp.tile([128, 16 * TPC], F32)
                for u in range(TPC):
                    rhs = xth[u // 8][:, (u % 8) * TILE : (u % 8 + 1) * TILE]
                    psumH = php.tile([105, TILE], F32)
                    nc.tensor.matmul(psumH, lhsT=L1, rhs=rhs, start=True, stop=True)
                    hg = hpool.tile([105, TILE], F32R)
                    if u % 3 < 2:
                        nc.scalar.activation(hg, psumH, AF.Relu)
                    else:
                        nc.vector.tensor_scalar_max(hg, psumH, 0.0)
                    for q in range(4):
                        nc.tensor.matmul(
                            psumM[:, 16 * u + 4 * q : 16 * u + 4 * q + 4],
                            lhsT=hg[:, 128 * q : 128 * q + 128],
                            rhs=L2p,
                            start=True,
                            stop=True,
                        )
                mst = mpool.tile([128, 16 * TPC], F32)
                if c % 2 == 0:
                    nc.scalar.activation(mst, psumM, AF.Copy)
                else:
                    nc.vector.tensor_copy(mst, psumM)
                nc.sync.dma_start(out=mx[c], in_=mst)
    nc.compile()
    return nc


def make_const_blobs(W1, b1, W2, b2, Wg, bg):
    cb = np.zeros((128, 256), np.float32)
    for e in range(E):
        for i in range(IN):
            cb[3 * e + i, 20 * e : 20 * e + 20] = W1[e, i, :]
        cb[15 + e, 20 * e : 20 * e + 20] = b1[e, :]
        cb[15 + e, 100 + e] = 1.0
        cb[20 * e : 20 * e + 20, 105:108] = W2[e, :, :]
        cb[100 + e, 105:108] = b2[e, :]
    for i in range(IN):
        for e in range(E):
            cb[:, 172 + 5 * i + e] = Wg[i, e]
    cb[:, 187:192] = bg
    cbr = np.ascontiguousarray(cb[0:105, 0:172])
    return cb, cbr


def assemble_mixed(mx):
    """mx [nchunk, 128, 12*TPC] -> mixed rows [nchunk*CHUNK, 3].

    mx[c, p, 16u+4q+o] is mixed component o of token
    c*CHUNK + 512u + 128q + p (o=3 is padding)."""
    nchunk = mx.shape[0]
    m = mx.reshape(nchunk, 128, TPC, 4, 4)[:, :, :, :, :3]
    return np.ascontiguousarray(m.transpose(0, 2, 3, 1, 4)).reshape(nchunk * CHUNK, 3)


def kernel(x, W1, b1, W2, b2, Wg, bg):
    x = np.ascontiguousarray(np.asarray(x, np.float32))
    bc = x.shape[0] // NCORES
    if bc not in _NC_CACHE:
        _NC_CACHE[bc] = build_nc(bc)
    nc = _NC_CACHE[bc]
    cb, cbr = make_const_blobs(
        np.asarray(W1), np.asarray(b1), np.asarray(W2), np.asarray(b2),
        np.asarray(Wg), np.asarray(bg),
    )
    in_maps = [
        {"x": x[c * bc : (c + 1) * bc], "cb": cb, "cbr": cbr}
        for c in range(NCORES)
    ]
    res = run_bass_kernel_spmd(nc, in_maps, list(range(NCORES)))
    gate = np.concatenate([res.results[c]["gate"] for c in range(NCORES)], axis=0)
    mixed = np.concatenate(
        [assemble_mixed(res.results[c]["mx"]) for c in range(NCORES)], axis=0
    )
    return mixed, gate
